# revision 1
# baseline (speedup 1.0000x reference)
"""CenterLoss (segment-reduce) kernel for Trainium2, 8 NeuronCores.

Math: out = (1/B) * sum_j sums_j / (counts_j * F)  over classes j with
counts_j > 0, where sums_j = sum_{i: label_i=j} ||feat_i - center_j||^2.

Strategy (data-parallel over batch, centers replicated):
  - Each core gets an 8192-sample shard. Features stream in as
    [128 part, blk, 512] tiles; the matching center row for every sample is
    fetched with the GPSIMD dma_gather ucode instruction (SWDGE) into the
    identical layout, so diff/square/reduce are plain elementwise ops:
      DVE:  diff = f - c                       [128, 512]
      ACT:  square + free-dim accumulate -> d  [128, 1]
  - Segment-reduce of d into 1000 class bins uses the factorization
    class = 32*q + r. Per 128-sample block, DVE builds one-hot(q) [128,32]
    and rhs = [one-hot(r)*d | one-hot(r)] [128,64] via tensor_scalar
    is_equal against an iota row; a single PE matmul per block accumulates
      psum[q, r]      += sum_i 1[q_i=q] * d_i * 1[r_i=r]   (sums)
      psum[q, 32+r]   += sum_i 1[q_i=q] * 1[r_i=r]         (counts)
    into one [32, 64] PSUM tile over all 64 blocks.
  - Host reduces the 8 per-core [32, 64] partials and applies the final
    division. Bins 1000..1023 stay zero.
"""

import os
from contextlib import ExitStack

import numpy as np

import concourse.bacc as bacc
import concourse.tile as tile
from concourse import mybir
from concourse.bass_utils import run_bass_kernel_spmd

NCORES = 8
BATCH = 65536
FEAT = 512
NCLASS = 1000
SHARD = BATCH // NCORES  # 8192
P = 128
NBLK = SHARD // P  # 64
CHUNK_BLKS = 8  # blocks per DMA chunk (1024 samples, 2 MB fp32)
NCHUNK = NBLK // CHUNK_BLKS
QW = 32  # class = QW*q + r, q,r in [0,32); 32*32 = 1024 bins >= 1000

# Dtype knobs: "f32" or "bf16" for the streamed features / gathered centers.
FEAT_DT = os.environ.get("CL_FEAT_DT", "f32")
CENT_DT = os.environ.get("CL_CENT_DT", "f32")

TRACE = os.environ.get("CL_TRACE", "0") == "1"

_DT = {"f32": mybir.dt.float32, "bf16": mybir.dt.bfloat16}
_NPDT = {"f32": np.float32, "bf16": None}  # bf16 via ml_dtypes below


def _np_dt(name):
    if name == "f32":
        return np.float32
    import ml_dtypes

    return ml_dtypes.bfloat16


def build_module():
    """Build + schedule + compile the per-core Bass program (SPMD: all 8
    cores run this same NEFF on their own shard)."""
    f32 = mybir.dt.float32
    i16 = mybir.dt.int16
    fdt = _DT[FEAT_DT]
    cdt = _DT[CENT_DT]

    nc = bacc.Bacc(
        "TRN2", target_bir_lowering=False, debug=False, num_devices=NCORES
    )
    feat_d = nc.dram_tensor("features", [SHARD, FEAT], fdt, kind="ExternalInput")
    cent_d = nc.dram_tensor("centers", [NCLASS, FEAT], cdt, kind="ExternalInput")
    # gather indices, wrapped-16 layout replicated over the 8 Q7 cores
    idx_d = nc.dram_tensor("labels16", [P, SHARD // 16], i16, kind="ExternalInput")
    # per-sample q/r in block layout: [p, b] = label[b*128+p] {//,%} 32
    q_d = nc.dram_tensor("qcol", [P, NBLK], f32, kind="ExternalInput")
    r_d = nc.dram_tensor("rcol", [P, NBLK], f32, kind="ExternalInput")
    iota_d = nc.dram_tensor("iota", [P, QW], f32, kind="ExternalInput")
    out_d = nc.dram_tensor("out", [QW, 2 * QW], f32, kind="ExternalOutput")

    with tile.TileContext(nc) as tc:
        with ExitStack() as ctx:
            singles = ctx.enter_context(tc.tile_pool(name="singles", bufs=1))
            fpool = ctx.enter_context(tc.tile_pool(name="fpool", bufs=3))
            gpool = ctx.enter_context(tc.tile_pool(name="gpool", bufs=3))
            dpool = ctx.enter_context(tc.tile_pool(name="dpool", bufs=4))
            sqpool = ctx.enter_context(tc.tile_pool(name="sqpool", bufs=4))
            small = ctx.enter_context(tc.tile_pool(name="small", bufs=6))
            psum_p = ctx.enter_context(
                tc.tile_pool(name="psum", bufs=1, space="PSUM")
            )

            idx_t = singles.tile([P, SHARD // 16], i16)
            nc.sync.dma_start(out=idx_t[:], in_=idx_d.ap())
            q_t = singles.tile([P, NBLK], f32)
            nc.sync.dma_start(out=q_t[:], in_=q_d.ap())
            r_t = singles.tile([P, NBLK], f32)
            nc.sync.dma_start(out=r_t[:], in_=r_d.ap())
            iota_t = singles.tile([P, QW], f32)
            nc.sync.dma_start(out=iota_t[:], in_=iota_d.ap())

            psum_t = psum_p.tile([QW, 2 * QW], f32, space="PSUM")
            feat_ap = feat_d.ap().rearrange("(b p) f -> p b f", p=P)

            nidx = CHUNK_BLKS * P  # gather indices per chunk
            for c in range(NCHUNK):
                ft = fpool.tile([P, CHUNK_BLKS, FEAT], fdt)
                nc.sync.dma_start(
                    out=ft[:],
                    in_=feat_ap[:, c * CHUNK_BLKS : (c + 1) * CHUNK_BLKS, :],
                )
                gt = gpool.tile([P, CHUNK_BLKS, FEAT], cdt)
                nc.gpsimd.dma_gather(
                    out_ap=gt[:],
                    in_ap=cent_d.ap(),
                    idxs_ap=idx_t[:, c * (nidx // 16) : (c + 1) * (nidx // 16)],
                    num_idxs=nidx,
                    num_idxs_reg=nidx,
                    elem_size=FEAT,
                )
                for j in range(CHUNK_BLKS):
                    b = c * CHUNK_BLKS + j
                    diff = dpool.tile([P, FEAT], f32)
                    nc.vector.tensor_tensor(
                        out=diff[:],
                        in0=ft[:, j, :],
                        in1=gt[:, j, :],
                        op=mybir.AluOpType.subtract,
                    )
                    sq = sqpool.tile([P, FEAT], f32)
                    dcol = small.tile([P, 1], f32)
                    nc.scalar.activation(
                        out=sq[:],
                        in_=diff[:],
                        func=mybir.ActivationFunctionType.Square,
                        accum_out=dcol[:],
                    )
                    ohq = small.tile([P, QW], f32)
                    nc.vector.tensor_scalar(
                        out=ohq[:],
                        in0=iota_t[:],
                        scalar1=q_t[:, b : b + 1],
                        scalar2=None,
                        op0=mybir.AluOpType.is_equal,
                    )
                    rhs = small.tile([P, 2 * QW], f32)
                    nc.vector.tensor_scalar(
                        out=rhs[:, QW:],
                        in0=iota_t[:],
                        scalar1=r_t[:, b : b + 1],
                        scalar2=None,
                        op0=mybir.AluOpType.is_equal,
                    )
                    nc.vector.tensor_scalar(
                        out=rhs[:, :QW],
                        in0=iota_t[:],
                        scalar1=r_t[:, b : b + 1],
                        scalar2=dcol[:],
                        op0=mybir.AluOpType.is_equal,
                        op1=mybir.AluOpType.mult,
                    )
                    nc.tensor.matmul(
                        out=psum_t[:],
                        lhsT=ohq[:],
                        rhs=rhs[:],
                        start=(b == 0),
                        stop=(b == NBLK - 1),
                    )
            res_t = singles.tile([QW, 2 * QW], f32)
            nc.vector.tensor_copy(out=res_t[:], in_=psum_t[:])
            nc.sync.dma_start(out=out_d.ap(), in_=res_t[:])

    nc.compile()
    return nc


_MODULE = None


def _get_module():
    global _MODULE
    if _MODULE is None:
        _MODULE = build_module()
    return _MODULE


def make_in_maps(features, centers, labels):
    """Host-side shard + layout prep. Returns list of 8 per-core input maps."""
    fdt = _np_dt(FEAT_DT)
    cdt = _np_dt(CENT_DT)
    features = np.ascontiguousarray(np.asarray(features), dtype=np.float32)
    centers = np.ascontiguousarray(np.asarray(centers), dtype=np.float32)
    labels = np.asarray(labels).astype(np.int64, copy=False)
    if fdt is not np.float32:
        features = features.astype(fdt)
    if cdt is not np.float32:
        centers = centers.astype(cdt)

    iota = np.ascontiguousarray(
        np.broadcast_to(np.arange(QW, dtype=np.float32), (P, QW))
    )
    in_maps = []
    for c in range(NCORES):
        lab = labels[c * SHARD : (c + 1) * SHARD]
        # wrapped-16 gather index layout: idx16[i % 16, i // 16] = lab[i],
        # replicated across the 8 groups of 16 partitions.
        idx16 = np.ascontiguousarray(lab.reshape(SHARD // 16, 16).T).astype(np.int16)
        idx16 = np.ascontiguousarray(np.tile(idx16, (8, 1)))
        lab_blk = lab.reshape(NBLK, P).T  # [p, b] = lab[b*128+p]
        in_maps.append(
            {
                "features": features[c * SHARD : (c + 1) * SHARD],
                "centers": centers,
                "labels16": idx16,
                "qcol": np.ascontiguousarray((lab_blk // QW).astype(np.float32)),
                "rcol": np.ascontiguousarray((lab_blk % QW).astype(np.float32)),
                "iota": iota,
            }
        )
    return in_maps


def reduce_outputs(outs):
    """Combine per-core [32, 64] partials into the scalar loss."""
    tot = np.sum(np.asarray(outs, dtype=np.float64), axis=0)  # [32, 64]
    sums = tot[:, :QW].reshape(-1)[:NCLASS]
    counts = tot[:, QW:].reshape(-1)[:NCLASS]
    per_class = np.where(counts > 0, sums / np.maximum(counts * FEAT, 1.0), 0.0)
    return np.asarray(per_class.sum() / BATCH, dtype=np.float32)


LAST_RESULT = None


def kernel(features, centers, labels):
    global LAST_RESULT
    nc = _get_module()
    in_maps = make_in_maps(features, centers, labels)
    res = run_bass_kernel_spmd(
        nc, in_maps, core_ids=list(range(NCORES)), trace=TRACE
    )
    LAST_RESULT = res
    outs = [r["out"] for r in res.results]
    return reduce_outputs(outs)
